# revision 5
# baseline (speedup 1.0000x reference)
"""Trainium2 Bass kernel for nn_MixedConvWithReLU — 1-D Winograd F(2,3).

Same problem/epilogue as kernel.py, but the kw dimension of the 3x3 conv is
Winograd-transformed: for each output column pair, 4 transform points replace
6 kw-tap row-streams (2/3 the PE rows; the machine is stream-rate bound).

  x~_0 = d0 - d2   x~_1 = d1 + d2   x~_2 = d2 - d1   x~_3 = d1 - d3
  (d_i = padded x cols 2q+i), computed once per image on DVE (fp16 out).
  m_pt[r, q] = sum_{h, kh} GW[pt, kh]^T @ x~_pt[r+kh, q]   (24 matmuls of
  [128]x[128, 8*28=224] per row-group per branch, accumulated in PSUM)
  y[2q]   = m0 + m1 + m2
  y[2q+1] = m1 - m2 - m3      (4 DVE tensor_tensor ops per row-group)

GW point weights stay exact in fp16 for the quantized branches:
  branch 0: w in {-1,0,1} -> GW in {0,+-0.5,+-1,+-1.5}
  branch 1: ints in [-7,7] -> halves of ints <= 21/2
  branch 2: BN-folded fp32 -> fp16 RNE (continuous act branch)
Predicted end-to-end rel err 7.3e-3 (numpy sim) vs the 2e-2 gate.
"""
import numpy as np
import concourse.bacc as bacc
import concourse.tile as tile
import concourse.mybir as mybir
from concourse.bass_utils import run_bass_kernel_spmd

F32 = mybir.dt.float32
F16 = mybir.dt.float16
I32 = mybir.dt.int32
AF = mybir.ActivationFunctionType
ALU = mybir.AluOpType

N_CORES = 8
B, CIN, COUT, H, W, K = 32, 256, 256, 56, 56, 3
B_PER = B // N_CORES          # 4 images per core
RG = 8                        # rows per psum tile
N_RG = H // RG                # 7 row-groups
HP = H + 2                    # padded 58
NQ = W // 2                   # 28 output column pairs
C_MAGIC = np.float32(1.5 * 2**23)
EPS = 1e-5

_cache = {}


def _quant_int(Wb, bits):
    Wb = Wb.astype(np.float32)
    levels = 2 ** (bits - 1) - 1
    step = np.float32(np.max(np.abs(Wb)) / np.float32(levels))
    return np.round(Wb / step).astype(np.float32), np.float64(step)


def _build(loop=True):
    nc = bacc.Bacc(trn_type="TRN2", debug=False)
    xr = nc.dram_tensor("xr", [B_PER, CIN, H, W], F16, kind="ExternalInput").ap()
    wr = nc.dram_tensor("wr", [128, 2 * 3 * 4 * 6 * 128], F16,
                        kind="ExternalInput").ap()
    cst = nc.dram_tensor("cst", [128, 12], F32, kind="ExternalInput").ap()
    iters = nc.dram_tensor("iters", [1, 1], I32, kind="ExternalInput").ap()
    out = nc.dram_tensor("out", [B_PER, COUT, H, W], F32, kind="ExternalOutput").ap()

    with tile.TileContext(nc) as tc:
        with (
            tc.tile_pool(name="fix", bufs=1) as fix,
            tc.tile_pool(name="ps", bufs=8, space="PSUM") as ps,
            tc.tile_pool(name="stage", bufs=3) as stage,
        ):
            wsb = fix.tile([128, 2, 3, 4, 6, 128], F16, tag="wsb")
            cst_t = fix.tile([128, 12], F32, tag="cst")
            xp = [fix.tile([128, 2, HP, HP], F16, tag=f"xp{s}", name=f"xp{s}")
                  for s in range(2)]
            # x~ per slot: [pt, h, row, quad]
            xt = [fix.tile([128, 4, 2, HP, NQ], F16, tag=f"xt{s}", name=f"xt{s}")
                  for s in range(2)]

            nc.sync.dma_start(
                out=wsb[:].rearrange("p h kh pt b m -> p (h kh pt b m)"), in_=wr)
            nc.sync.dma_start(out=cst_t[:], in_=cst)

            if loop:
                tmp = nc.alloc_registers("iters_reg", mybir.ALL_ENGINES)
                nc.regs_load(tmp, iters[0:1, 0:1])
                n_it = nc.snap(tmp, donate=True, min_val=1, max_val=1000000)

            for s in range(2):
                nc.vector.memset(xp[s][:], 0.0)

            from contextlib import nullcontext
            with (tc.For_i(0, n_it, 1) if loop else nullcontext()):
                if loop:
                    nc.gpsimd.nop()
                for img in range(B_PER):
                    s = img % 2
                    for h in range(2):
                        # gpsimd queue: otherwise the issue slot queues behind
                        # the previous image's epilogue ACTs on scalar
                        nc.gpsimd.dma_start(
                            out=xp[s][:, h, 1:H + 1, 1:W + 1],
                            in_=xr[img, 128 * h:128 * (h + 1), :, :])
                    # input transform: d_i = padded cols 2q+i via (q t) split
                    for h in range(2):
                        xq = xp[s][:, h].rearrange("p r (q t) -> p r q t", t=2)
                        d0 = xq[:, :, 0:NQ, 0]
                        d1 = xq[:, :, 0:NQ, 1]
                        d2 = xq[:, :, 1:NQ + 1, 0]
                        d3 = xq[:, :, 1:NQ + 1, 1]
                        nc.vector.tensor_tensor(
                            out=xt[s][:, 0, h], in0=d0, in1=d2, op=ALU.subtract)
                        nc.vector.tensor_tensor(
                            out=xt[s][:, 1, h], in0=d1, in1=d2, op=ALU.add)
                        nc.vector.tensor_tensor(
                            out=xt[s][:, 2, h], in0=d2, in1=d1, op=ALU.subtract)
                        nc.vector.tensor_tensor(
                            out=xt[s][:, 3, h], in0=d1, in1=d3, op=ALU.subtract)
                    for j in range(2):
                        accs = {}
                        a3s = {}
                        for br in range(3):
                            blk = 2 * br + j
                            for r in range(N_RG):
                                r0 = RG * r
                                m = [ps.tile([128, RG, NQ], F32, tag="ps",
                                             name=f"m{pt}") for pt in range(4)]
                                for pt in range(4):
                                    n = 0
                                    for h in range(2):
                                        for kh in range(3):
                                            nc.tensor.matmul(
                                                out=m[pt][:],
                                                lhsT=wsb[:, h, kh, pt, blk, :],
                                                rhs=xt[s][:, pt, h,
                                                          r0 + kh:r0 + kh + RG, :],
                                                start=(n == 0), stop=(n == 5))
                                            n += 1
                                # output transform -> y [128, RG, 56]
                                # (DVE may read at most one PSUM operand per
                                # op: stage m1 in SBUF, then chain)
                                y = stage.tile([128, RG, W], F32, tag="y",
                                               name="y", bufs=6)
                                yq = y[:].rearrange("p r (q t) -> p r q t", t=2)
                                m1s = stage.tile([128, RG, NQ], F32, tag="m1s",
                                                 name="m1s")
                                # PSUM->SBUF copy on ScalarE (has slack; DVE
                                # is the busier engine here)
                                nc.scalar.activation(
                                    out=m1s[:], in_=m[1][:], func=AF.Identity,
                                    bias=0.0, scale=1.0)
                                t01 = stage.tile([128, RG, NQ], F32, tag="t01",
                                                 name="t01")
                                nc.vector.tensor_tensor(
                                    out=t01[:], in0=m[0][:], in1=m1s[:],
                                    op=ALU.add)
                                nc.vector.tensor_tensor(
                                    out=yq[:, :, :, 0], in0=t01[:], in1=m[2][:],
                                    op=ALU.add)
                                t13 = stage.tile([128, RG, NQ], F32, tag="t13",
                                                 name="t13")
                                nc.vector.tensor_tensor(
                                    out=t13[:], in0=m1s[:], in1=m[3][:],
                                    op=ALU.subtract)
                                nc.vector.tensor_tensor(
                                    out=yq[:, :, :, 1], in0=t13[:], in1=m[2][:],
                                    op=ALU.subtract)

                                if br == 0:
                                    acc = stage.tile([128, RG, W], F32,
                                                     tag="acc", name="acc", bufs=8)
                                    nc.vector.tensor_scalar(
                                        out=acc[:], in0=y[:],
                                        scalar1=cst_t[:, 0 + j:1 + j],
                                        scalar2=cst_t[:, 6:7],
                                        op0=ALU.is_gt, op1=ALU.mult)
                                    accs[r] = acc
                                elif br == 1:
                                    t4 = stage.tile([128, RG, W], F32,
                                                    tag="t4", name="t4")
                                    nc.scalar.activation(
                                        out=t4[:], in_=y[:], func=AF.Identity,
                                        bias=cst_t[:, 2 + j:3 + j],
                                        scale=cst_t[:, 10 + j:11 + j])
                                    u4 = stage.tile([128, RG, W], F32,
                                                    tag="u4", name="u4")
                                    nc.vector.tensor_scalar(
                                        out=u4[:], in0=t4[:],
                                        scalar1=float(C_MAGIC),
                                        scalar2=float(C_MAGIC + 15.0),
                                        op0=ALU.add, op1=ALU.min)
                                    q4 = stage.tile([128, RG, W], F32,
                                                    tag="q4", name="q4")
                                    nc.vector.tensor_scalar(
                                        out=q4[:], in0=u4[:],
                                        scalar1=float(C_MAGIC),
                                        scalar2=float(C_MAGIC),
                                        op0=ALU.max, op1=ALU.subtract)
                                    a3 = stage.tile([128, RG, W], F32,
                                                    tag="a3", name="a3", bufs=8)
                                    nc.vector.scalar_tensor_tensor(
                                        out=a3[:], in0=q4[:], scalar=cst_t[:, 7:8],
                                        in1=accs.pop(r)[:], op0=ALU.mult,
                                        op1=ALU.add)
                                    a3s[r] = a3
                                else:
                                    y16 = stage.tile([128, RG, W], F32,
                                                     tag="y16", name="y16")
                                    nc.scalar.activation(
                                        out=y16[:], in_=y[:], func=AF.Relu,
                                        bias=cst_t[:, 4 + j:5 + j], scale=1.0)
                                    v16 = stage.tile([128, RG, W], F32,
                                                     tag="v16", name="v16")
                                    nc.vector.tensor_scalar(
                                        out=v16[:], in0=y16[:],
                                        scalar1=cst_t[:, 8:9],
                                        scalar2=cst_t[:, 9:10],
                                        op0=ALU.mult, op1=ALU.min)
                                    o = stage.tile([128, RG, W], F32,
                                                   tag="o", name="o")
                                    nc.vector.tensor_tensor(
                                        out=o[:], in0=a3s.pop(r)[:],
                                        in1=v16[:], op=ALU.add)
                                    nc.sync.dma_start(
                                        out=out[img, 128 * j:128 * (j + 1),
                                                r0:r0 + RG, :],
                                        in_=o[:])
                if loop:
                    nc.gpsimd.nop()

    nc.compile()
    return nc


def _prepare(x, Wt, bn_gamma, bn_beta, bn_mean, bn_var, alphas):
    x = np.ascontiguousarray(x, np.float32)
    Wt = np.asarray(Wt, np.float32)
    a64 = np.asarray(alphas, np.float64)
    e = np.exp(a64 - a64.max())
    wsoft = (e / e.sum()).astype(np.float64)
    w0, w1, w2 = wsoft

    inv = (np.asarray(bn_gamma, np.float64)
           / np.sqrt(np.asarray(bn_var, np.float64) + EPS))
    bias = (np.asarray(bn_beta, np.float64)
            - np.asarray(bn_mean, np.float64) * inv)

    scale0 = np.float64(np.mean(np.abs(Wt[0]), dtype=np.float32))
    Wdev = [np.sign(Wt[0]).astype(np.float64), None, None]
    k1, step1 = _quant_int(Wt[1], 4)
    Wdev[1] = k1.astype(np.float64)
    k2, step2 = _quant_int(Wt[2], 16)
    Wdev[2] = (k2.astype(np.float64) * step2
               * inv[2][:, None, None, None])

    # GW points per branch: [4][Cout, Cin, 3(kh)]
    Whost = np.empty((128, 2, 3, 4, 6, 128), np.float16)
    for i in range(3):
        w = Wdev[i]                                   # [Cout, Cin, 3, 3] f64
        g = [w[:, :, :, 0],
             (w[:, :, :, 0] + w[:, :, :, 1] + w[:, :, :, 2]) / 2,
             (w[:, :, :, 0] - w[:, :, :, 1] + w[:, :, :, 2]) / 2,
             w[:, :, :, 2]]
        for j in range(2):
            blk = 2 * i + j
            for pt in range(4):
                sub = g[pt][128 * j:128 * (j + 1), :, :]   # [128m, 256, 3]
                for h in range(2):
                    # -> [cin_p, kh, cout_m]
                    Whost[:, h, :, pt, blk, :] = \
                        sub[:, 128 * h:128 * (h + 1), :].transpose(1, 2, 0) \
                        .astype(np.float16)
    wr = Whost.reshape(128, -1)

    cst = np.zeros((128, 12), np.float32)
    for j in range(2):
        sl = slice(128 * j, 128 * (j + 1))
        cst[:, 0 + j] = ((0.5 - bias[0][sl]) / (inv[0][sl] * scale0)) \
            .astype(np.float32)
        cst[:, 2 + j] = (bias[1][sl] * 15.0).astype(np.float32)
        cst[:, 4 + j] = bias[2][sl].astype(np.float32)
        cst[:, 10 + j] = (step1 * inv[1][sl] * 15.0).astype(np.float32)
    cst[:, 6] = np.float32(w0)
    cst[:, 7] = np.float32(w1 / 15.0)
    cst[:, 8] = np.float32(w2)
    cst[:, 9] = np.float32(w2)

    x16 = x.astype(np.float16)
    return x16, wr, cst


def kernel(x, W, bn_gamma, bn_beta, bn_mean, bn_var, alphas, _iters=1,
           variant="wino"):
    loop = _iters > 1
    key = ("wino", loop)
    if key not in _cache:
        _cache[key] = _build(loop=loop)
    nc = _cache[key]
    x16, wr, cst = _prepare(x, W, bn_gamma, bn_beta, bn_mean, bn_var, alphas)

    it = np.array([[_iters]], np.int32)
    in_maps = [
        {"xr": x16[B_PER * c:B_PER * (c + 1)], "wr": wr, "cst": cst, "iters": it}
        for c in range(N_CORES)
    ]
    res = run_bass_kernel_spmd(nc, in_maps, list(range(N_CORES)))
    outs = [res.results[c]["out"] for c in range(N_CORES)]
    return np.concatenate(outs, axis=0)
